# revision 9
# baseline (speedup 1.0000x reference)
"""Causal multi-head attention block on 8 Trainium2 NeuronCores.

Problem: x[4,2048,1024] -> qkv proj (16 heads, dh=64) -> causal softmax
attention -> out proj. Sharding: core = (batch, head-half): each core
computes QKV for 8 heads of one batch, attention for those heads, and a
partial O-projection (its 512 input columns of W_o); host sums the two
partials per batch.

Device kernel (identical SPMD program, per-core data):
  - layouts: x.T [d, t] (host pre-transposed), Q.T/K.T computed as
    [o, t] (feature-major), V as [t, o] with a ones-column appended.
  - scores computed transposed: S.T[k_tile, q_span] = K.T_blk^T @ Q.T,
    exp on ScalarE (scale=1/8 folded in; scores are O(1) so no
    max-subtraction needed), diagonal blocks masked with a 0/1
    lower-triangle multiply after exp.
  - P.T @ [V | 1] with P.T stationary uses the full 128x128 PE array and
    accumulates both numerator and softmax denominator in one PSUM tile.
  - O normalized, transposed on PE, then O-proj partial + 0.5*b_o.

All matmuls bf16 with fp32 PSUM accumulation.
"""

import numpy as np
import ml_dtypes

BF16 = ml_dtypes.bfloat16

B, T, D = 4, 2048, 1024
NH, DH = 16, 64
HPC = 8            # heads per core
OC = HPC * DH      # 512: per-core head columns
NT = T // 128      # 16 q/k tiles of 128
ND = D // 128      # 8 d-tiles
N_CORES = 8

_cache = {}


def _build(debug=False):
    import concourse.bass as bass
    import concourse.mybir as mybir
    import concourse.tile as tile
    from concourse import bacc
    from concourse.masks import make_identity

    f32 = mybir.dt.float32
    bf16 = mybir.dt.bfloat16
    Exp = mybir.ActivationFunctionType.Exp

    nc = bacc.Bacc("TRN2", target_bir_lowering=False, debug=False,
                   num_devices=N_CORES)

    xT = nc.declare_dram_parameter("xT", [D, T], bf16, isOutput=False)
    wqk = nc.declare_dram_parameter("wqkT", [D, 2 * OC], bf16, isOutput=False)
    wv = nc.declare_dram_parameter("wvT", [D, OC], bf16, isOutput=False)
    wo = nc.declare_dram_parameter("woT", [OC, D], bf16, isOutput=False)
    bqk = nc.declare_dram_parameter("bqk", [2 * OC, 1], f32, isOutput=False)
    bv = nc.declare_dram_parameter("bv", [1, OC], f32, isOutput=False)
    bo = nc.declare_dram_parameter("bo", [1, D], f32, isOutput=False)
    tri = nc.declare_dram_parameter("tri", [128, 128], bf16, isOutput=False)
    out = nc.declare_dram_parameter("out", [T, D], f32, isOutput=True)
    if debug:
        d_qkt = nc.declare_dram_parameter("d_qkt", [128, ND * T], bf16, isOutput=True)
        d_vp = nc.declare_dram_parameter(
            "d_vp", [128, NT * HPC * (DH + 1)], bf16, isOutput=True)
        d_osb = nc.declare_dram_parameter(
            "d_osb", [128, NT * HPC * DH], bf16, isOutput=True)
        d_ot = nc.declare_dram_parameter(
            "d_ot", [128, (OC // 128) * T], bf16, isOutput=True)

    with tile.TileContext(nc) as tc:
        with (
            tc.tile_pool(name="persist", bufs=1) as persist,
            tc.tile_pool(name="pt", bufs=4) as ptp,
            tc.tile_pool(name="dn", bufs=4) as dnp,
            tc.tile_pool(name="ostage", bufs=3) as ostage,
            tc.tile_pool(name="psmm", bufs=2, space="PSUM") as psmm,
            tc.tile_pool(name="psS", bufs=2, space="PSUM") as psS,
            tc.tile_pool(name="psO", bufs=4, space="PSUM") as psO,
        ):
            # ---- persistent SBUF tensors ----
            XT = persist.tile([128, ND, T], bf16)          # x.T d-tiles
            WQK = persist.tile([128, ND, 2 * OC], bf16)
            WV = persist.tile([128, ND, OC], bf16)
            WO = persist.tile([128, OC // 128, D], bf16)
            BQK = persist.tile([128, ND, 1], f32)
            BV = persist.tile([128, OC], f32)
            BO = persist.tile([128, D], f32)
            TRI = persist.tile([128, 128], bf16)
            IDENT = persist.tile([128, 128], bf16)
            QKT = persist.tile([128, ND, T], bf16)         # [o, t] Q.T|K.T
            VP = persist.tile([128, NT, HPC, DH + 1], bf16)  # [k, h, V|1]
            OSB = persist.tile([128, NT, HPC, DH], bf16)   # attn out [q, c]
            OT = persist.tile([128, OC // 128, T], bf16)   # attn out.T [c, t]

            nc.sync.dma_start(out=XT[:], in_=xT.rearrange("(n p) t -> p n t", p=128))
            nc.sync.dma_start(out=WQK[:], in_=wqk.rearrange("(n p) o -> p n o", p=128))
            nc.sync.dma_start(out=WV[:], in_=wv.rearrange("(n p) o -> p n o", p=128))
            nc.sync.dma_start(out=WO[:], in_=wo.rearrange("(n p) o -> p n o", p=128))
            nc.sync.dma_start(out=BQK[:], in_=bqk.rearrange("(n p) o -> p n o", p=128))
            nc.gpsimd.dma_start(out=BV[:], in_=bv[:, :].to_broadcast((128, OC)))
            nc.gpsimd.dma_start(out=BO[:], in_=bo[:, :].to_broadcast((128, D)))
            nc.sync.dma_start(out=TRI[:], in_=tri[:, :])
            make_identity(nc, IDENT[:])
            nc.vector.memset(VP[:, :, :, DH:DH + 1], 1.0)

            # ---- QK.T: [o, t] = W_qk @ x.T  (o-tiles: 4 Q then 4 K) ----
            for ot in range(2 * OC // 128):
                for tch in range(T // 512):
                    ps = psmm.tile([128, 512], f32, tag="mm")
                    for kd in range(ND):
                        nc.tensor.matmul(
                            ps[:],
                            lhsT=WQK[:, kd, ot * 128:(ot + 1) * 128],
                            rhs=XT[:, kd, tch * 512:(tch + 1) * 512],
                            start=(kd == 0), stop=(kd == ND - 1),
                        )
                    nc.vector.tensor_scalar_add(
                        QKT[:, ot, tch * 512:(tch + 1) * 512], ps[:],
                        BQK[:, ot, 0:1],
                    )

            # ---- V: [t, o] = x @ W_v.T, bias, ones col stays 1 ----
            for tt in range(NT):
                ps = psmm.tile([128, OC], f32, tag="mm")
                for kd in range(ND):
                    nc.tensor.matmul(
                        ps[:],
                        lhsT=XT[:, kd, tt * 128:(tt + 1) * 128],
                        rhs=WV[:, kd, :],
                        start=(kd == 0), stop=(kd == ND - 1),
                    )
                for h in range(HPC):
                    nc.vector.tensor_tensor(
                        out=VP[:, tt, h, 0:DH],
                        in0=ps[:, h * DH:(h + 1) * DH],
                        in1=BV[:, h * DH:(h + 1) * DH],
                        op=mybir.AluOpType.add,
                    )

            # ---- attention per head ----
            for h in range(HPC):
                prow = (h % 2) * 64
                QTh = QKT[prow:prow + 64, h // 2, :]       # [64, T]
                KTh = QKT[prow:prow + 64, 4 + h // 2, :]   # [64, T]
                for J in range(T // 512):                  # q superblock
                    po = [psO.tile([128, DH + 1], f32, tag="o", name=f"po{h}_{J}_{jp}")
                          for jp in range(4)]
                    nk = 4 * J + 4
                    for i in range(nk):
                        qlo = max(J * 512, i * 128)
                        qhi = (J + 1) * 512
                        span = qhi - qlo
                        ps = psS.tile([128, 512], f32, tag="s")
                        nc.tensor.matmul(
                            ps[:, 0:span],
                            lhsT=KTh[:, i * 128:(i + 1) * 128],
                            rhs=QTh[:, qlo:qhi],
                            start=True, stop=True,
                        )
                        pt = ptp.tile([128, 512], bf16, tag="p")
                        nc.scalar.activation(
                            out=pt[:, 0:span], in_=ps[:, 0:span],
                            func=Exp, scale=0.125,
                        )
                        if i >= 4 * J:  # diagonal block: zero upper triangle
                            nc.vector.tensor_tensor(
                                out=pt[:, 0:128], in0=pt[:, 0:128], in1=TRI[:],
                                op=mybir.AluOpType.mult,
                            )
                        for jp in range((qlo - J * 512) // 128, 4):
                            qoff = J * 512 + jp * 128 - qlo
                            nc.tensor.matmul(
                                po[jp][:],
                                lhsT=pt[:, qoff:qoff + 128],
                                rhs=VP[:, i, h, :],
                                start=(i == 0), stop=(i == 4 * J + jp),
                            )
                    for jp in range(4):
                        den = dnp.tile([128, 1], f32, tag="d")
                        nc.vector.reciprocal(den[:], po[jp][:, DH:DH + 1])
                        nc.vector.tensor_scalar_mul(
                            OSB[:, 4 * J + jp, h, :], po[jp][:, 0:DH], den[:],
                        )

            # ---- transpose O: [q, c] -> [c, t] ----
            for ct in range(OC // 128):
                for tq in range(NT):
                    pst = psmm.tile([128, 128], bf16, tag="mm")
                    blk = OSB[:, tq, 2 * ct:2 * ct + 2, :].rearrange("p a b -> p (a b)")
                    nc.tensor.transpose(pst[:], blk, IDENT[:])
                    nc.vector.tensor_copy(OT[:, ct, tq * 128:(tq + 1) * 128], pst[:])

            # ---- O-proj partial: out = O @ WoT + 0.5 b_o ----
            for tq in range(NT):
                for oc2 in range(D // 512):
                    ps = psmm.tile([128, 512], f32, tag="mm")
                    for ct in range(OC // 128):
                        nc.tensor.matmul(
                            ps[:],
                            lhsT=OT[:, ct, tq * 128:(tq + 1) * 128],
                            rhs=WO[:, ct, oc2 * 512:(oc2 + 1) * 512],
                            start=(ct == 0), stop=(ct == OC // 128 - 1),
                        )
                    ob = ostage.tile([128, 512], f32, tag="ob")
                    nc.vector.tensor_tensor(
                        out=ob[:], in0=ps[:],
                        in1=BO[:, oc2 * 512:(oc2 + 1) * 512],
                        op=mybir.AluOpType.add,
                    )
                    nc.sync.dma_start(
                        out=out[tq * 128:(tq + 1) * 128, oc2 * 512:(oc2 + 1) * 512],
                        in_=ob[:],
                    )

            if debug:
                nc.sync.dma_start(
                    out=d_qkt[:, :], in_=QKT[:].rearrange("p a t -> p (a t)"))
                nc.sync.dma_start(
                    out=d_vp[:, :], in_=VP[:].rearrange("p a b c -> p (a b c)"))
                nc.sync.dma_start(
                    out=d_osb[:, :], in_=OSB[:].rearrange("p a b c -> p (a b c)"))
                nc.sync.dma_start(
                    out=d_ot[:, :], in_=OT[:].rearrange("p a t -> p (a t)"))

    nc.compile()
    return nc


def _in_maps(x, W_qkv, b_qkv, W_o, b_o):
    x = np.asarray(x, np.float32)
    W_qkv = np.asarray(W_qkv, np.float32)
    b_qkv = np.asarray(b_qkv, np.float32)
    W_o = np.asarray(W_o, np.float32)
    b_o = np.asarray(b_o, np.float32)

    maps = []
    for c in range(N_CORES):
        b, hh = c // 2, c % 2
        rs = slice(hh * OC, (hh + 1) * OC)
        wq = W_qkv[0 * D:1 * D][rs]            # [512, 1024]
        wk = W_qkv[1 * D:2 * D][rs]
        wv = W_qkv[2 * D:3 * D][rs]
        wqkT = np.concatenate([wq, wk], 0).T   # [1024, 1024]
        bq = b_qkv[0 * D:1 * D][rs]
        bk = b_qkv[1 * D:2 * D][rs]
        bvv = b_qkv[2 * D:3 * D][rs]
        tri = np.triu(np.ones((128, 128), np.float32))
        maps.append({
            "xT": np.ascontiguousarray(x[b].T).astype(BF16),
            "wqkT": np.ascontiguousarray(wqkT).astype(BF16),
            "wvT": np.ascontiguousarray(wv.T).astype(BF16),
            "woT": np.ascontiguousarray(W_o[:, rs].T).astype(BF16),
            "bqk": np.concatenate([bq, bk]).reshape(2 * OC, 1),
            "bv": bvv.reshape(1, OC),
            "bo": (0.5 * b_o).reshape(1, D),
            "tri": tri.astype(BF16),
        })
    return maps


def _run(x, W_qkv, b_qkv, W_o, b_o, trace=False, tmpdir=None):
    from concourse.bass_utils import run_bass_kernel_spmd

    if "nc" not in _cache:
        _cache["nc"] = _build()
    res = run_bass_kernel_spmd(
        _cache["nc"], _in_maps(x, W_qkv, b_qkv, W_o, b_o),
        core_ids=list(range(N_CORES)), trace=trace, tmpdir=tmpdir,
    )
    out = np.empty((B, T, D), np.float32)
    for b in range(B):
        out[b] = res.results[2 * b]["out"] + res.results[2 * b + 1]["out"]
    return out, res


def kernel(x, W_qkv, b_qkv, W_o, b_o):
    out, _ = _run(x, W_qkv, b_qkv, W_o, b_o, trace=False)
    return out


# revision 16
# speedup vs baseline: 1.2275x; 1.2275x over previous
"""Causal multi-head attention block on 8 Trainium2 NeuronCores.

Problem: x[4,2048,1024] -> qkv proj (16 heads, dh=64) -> causal softmax
attention -> out proj. Sharding: core = (batch, head-half): each core
computes QKV for 8 heads of one batch, attention for those heads, and a
partial O-projection (its 512 input columns of W_o); host sums the two
partials per batch.

Device kernel (identical SPMD program, per-core data):
  - layouts: x.T [d, t] (host pre-transposed), Q.T/K.T computed as
    [o, t] (feature-major), V as [t, o] with a ones-column appended.
  - scores computed transposed: S.T[k_tile, q_span] = K.T_blk^T @ Q.T,
    exp on ScalarE (scale=1/8 folded in; scores are O(1) so no
    max-subtraction needed), diagonal blocks masked with a 0/1
    lower-triangle multiply after exp.
  - P.T @ [V | 1] with P.T stationary uses the full 128x128 PE array and
    accumulates both numerator and softmax denominator in one PSUM tile.
  - O normalized, transposed on PE, then O-proj partial + 0.5*b_o.

All matmuls bf16 with fp32 PSUM accumulation.
"""

import numpy as np
import ml_dtypes

BF16 = ml_dtypes.bfloat16

B, T, D = 4, 2048, 1024
NH, DH = 16, 64
HPC = 8            # heads per core
OC = HPC * DH      # 512: per-core head columns
NT = T // 128      # 16 q/k tiles of 128
ND = D // 128      # 8 d-tiles
N_CORES = 8

_cache = {}


def _build(debug=False):
    import concourse.bass as bass
    import concourse.mybir as mybir
    import concourse.tile as tile
    from concourse import bacc
    from concourse.masks import make_identity

    f32 = mybir.dt.float32
    bf16 = mybir.dt.bfloat16
    Exp = mybir.ActivationFunctionType.Exp

    nc = bacc.Bacc("TRN2", target_bir_lowering=False, debug=False,
                   num_devices=N_CORES)

    xT = nc.declare_dram_parameter("xT", [D, T], bf16, isOutput=False)
    wqk = nc.declare_dram_parameter("wqkT", [D, 2 * OC], bf16, isOutput=False)
    wv = nc.declare_dram_parameter("wvT", [D, OC], bf16, isOutput=False)
    wo = nc.declare_dram_parameter("woT", [OC, D], bf16, isOutput=False)
    bqk = nc.declare_dram_parameter("bqk", [2 * OC, 1], f32, isOutput=False)
    bv = nc.declare_dram_parameter("bv", [1, OC], f32, isOutput=False)
    bo = nc.declare_dram_parameter("bo", [1, D], f32, isOutput=False)
    tri = nc.declare_dram_parameter("tri", [128, 128], bf16, isOutput=False)
    out = nc.declare_dram_parameter("out", [T, D], f32, isOutput=True)
    if debug:
        d_qkt = nc.declare_dram_parameter("d_qkt", [128, ND * T], bf16, isOutput=True)
        d_vp = nc.declare_dram_parameter(
            "d_vp", [128, NT * HPC * (DH + 1)], bf16, isOutput=True)
        d_osb = nc.declare_dram_parameter(
            "d_osb", [128, NT * HPC * DH], bf16, isOutput=True)
        d_ot = nc.declare_dram_parameter(
            "d_ot", [128, (OC // 128) * T], bf16, isOutput=True)

    with tile.TileContext(nc) as tc:
        with (
            tc.tile_pool(name="persist", bufs=1) as persist,
            tc.tile_pool(name="pt", bufs=4) as ptp,
            tc.tile_pool(name="dn", bufs=4) as dnp,
            tc.tile_pool(name="ostage", bufs=3) as ostage,
            tc.tile_pool(name="psS", bufs=2, space="PSUM") as psS,
            tc.tile_pool(name="psO", bufs=4, space="PSUM") as psO,
        ):
            # ---- persistent SBUF tensors ----
            XT = persist.tile([128, ND, T], bf16)          # x.T d-tiles
            WQK = persist.tile([128, ND, 2 * OC], bf16)
            WV = persist.tile([128, ND, OC], bf16)
            WO = persist.tile([128, OC // 128, D], bf16)
            BQK = persist.tile([128, ND, 1], f32)
            BV = persist.tile([128, OC], f32)
            BO = persist.tile([128, D], f32)
            TRI = persist.tile([128, 128], bf16)
            IDENT = persist.tile([128, 128], bf16)
            QKT = persist.tile([128, ND, T], bf16)         # [o, t] Q.T|K.T
            VP = persist.tile([128, NT, HPC, DH + 1], bf16)  # [k, h, V|1]
            OSB = persist.tile([128, NT, HPC, DH], bf16)   # attn out [q, c]
            OT = persist.tile([128, OC // 128, T], bf16)   # attn out.T [c, t]

            nc.sync.dma_start(out=XT[:], in_=xT.rearrange("(n p) t -> p n t", p=128))
            nc.sync.dma_start(out=WQK[:], in_=wqk.rearrange("(n p) o -> p n o", p=128))
            nc.sync.dma_start(out=WV[:], in_=wv.rearrange("(n p) o -> p n o", p=128))
            nc.sync.dma_start(out=WO[:], in_=wo.rearrange("(n p) o -> p n o", p=128))
            nc.sync.dma_start(out=BQK[:], in_=bqk.rearrange("(n p) o -> p n o", p=128))
            nc.gpsimd.dma_start(out=BV[:], in_=bv[:, :].to_broadcast((128, OC)))
            nc.gpsimd.dma_start(out=BO[:], in_=bo[:, :].to_broadcast((128, D)))
            nc.sync.dma_start(out=TRI[:], in_=tri[:, :])
            make_identity(nc, IDENT[:])
            nc.vector.memset(VP[:, :, :, DH:DH + 1], 1.0)

            # ---- QK.T: [o, t] = W_qk @ x.T  (o-tiles: 4 Q then 4 K) ----
            for ot in range(2 * OC // 128):
                for tch in range(T // 512):
                    ps = psS.tile([128, 1024], f32, tag="s")
                    for kd in range(ND):
                        nc.tensor.matmul(
                            ps[:, 0:512],
                            lhsT=WQK[:, kd, ot * 128:(ot + 1) * 128],
                            rhs=XT[:, kd, tch * 512:(tch + 1) * 512],
                            start=(kd == 0), stop=(kd == ND - 1),
                        )
                    nc.vector.tensor_scalar_add(
                        QKT[:, ot, tch * 512:(tch + 1) * 512], ps[:, 0:512],
                        BQK[:, ot, 0:1],
                    )

            # ---- V: [t, o] = x @ W_v.T, bias, ones col stays 1 ----
            for tt in range(NT):
                ps = psS.tile([128, 1024], f32, tag="s")
                for kd in range(ND):
                    nc.tensor.matmul(
                        ps[:, 0:OC],
                        lhsT=XT[:, kd, tt * 128:(tt + 1) * 128],
                        rhs=WV[:, kd, :],
                        start=(kd == 0), stop=(kd == ND - 1),
                    )
                nc.vector.tensor_tensor(
                    out=VP[:, tt, :, 0:DH],
                    in0=ps[:, 0:OC].rearrange("p (a b) -> p a b", b=DH),
                    in1=BV[:].rearrange("p (a b) -> p a b", b=DH),
                    op=mybir.AluOpType.add,
                )

            # ---- attention per head; q superblocks of 1024 ----
            def transpose_ct(ct):
                # O[q, c] -> OT[c, t] for head pair (2ct, 2ct+1)
                for tq in range(NT):
                    pst = psS.tile([128, 1024], bf16, tag="s",
                                    name=f"pst{ct}_{tq}")
                    blk = OSB[:, tq, 2 * ct:2 * ct + 2, :].rearrange(
                        "p a b -> p (a b)")
                    nc.tensor.transpose(pst[:, 0:128], blk, IDENT[:])
                    nc.vector.tensor_copy(
                        OT[:, ct, tq * 128:(tq + 1) * 128], pst[:, 0:128])

            for h in range(HPC):
                prow = (h % 2) * 64
                QTh = QKT[prow:prow + 64, h // 2, :]       # [64, T]
                KTh = QKT[prow:prow + 64, 4 + h // 2, :]   # [64, T]
                for J in range(T // 512):                  # q superblock of 512
                    po = [psO.tile([128, DH + 1], f32, tag="o",
                                   name=f"po{h}_{J}_{jp}") for jp in range(4)]
                    ks = list(range(4 * J + 4))
                    pairs = [ks[m:m + 2] for m in range(0, len(ks), 2)]
                    for pair in pairs:
                        # pack S.T of both k-tiles side by side, one exp
                        ps = psS.tile([128, 1024], f32, tag="s")
                        pt = ptp.tile([128, 1024], bf16, tag="p")
                        col = 0
                        offs = []
                        for i in pair:
                            qlo = max(J * 512, i * 128)
                            span = (J + 1) * 512 - qlo
                            # each matmul region must stay within one bank
                            assert col // 512 == (col + span - 1) // 512
                            nc.tensor.matmul(
                                ps[:, col:col + span],
                                lhsT=KTh[:, i * 128:(i + 1) * 128],
                                rhs=QTh[:, qlo:qlo + span],
                                start=True, stop=True,
                            )
                            offs.append((i, col, qlo, span))
                            col += span
                        nc.scalar.activation(
                            out=pt[:, 0:col], in_=ps[:, 0:col],
                            func=Exp, scale=0.125,
                        )
                        for i, coff, qlo, span in offs:
                            if i >= 4 * J:  # diagonal: zero upper triangle
                                nc.vector.tensor_tensor(
                                    out=pt[:, coff:coff + 128],
                                    in0=pt[:, coff:coff + 128], in1=TRI[:],
                                    op=mybir.AluOpType.mult,
                                )
                            for jp in range((qlo - J * 512) // 128, 4):
                                qoff = coff + J * 512 + jp * 128 - qlo
                                nc.tensor.matmul(
                                    po[jp][:],
                                    lhsT=pt[:, qoff:qoff + 128],
                                    rhs=VP[:, i, h, :],
                                    start=(i == 0), stop=(i == 4 * J + jp),
                                )
                    for jp in range(4):
                        den = dnp.tile([128, 1], f32, tag="d")
                        nc.vector.reciprocal(den[:], po[jp][:, DH:DH + 1])
                        nc.vector.tensor_scalar_mul(
                            OSB[:, 4 * J + jp, h, :], po[jp][:, 0:DH], den[:],
                        )
                if h % 2 == 1:
                    transpose_ct(h // 2)

            # ---- O-proj partial: out = O @ WoT + 0.5 b_o ----
            for tq in range(NT):
                for oc2 in range(D // 512):
                    ps = psS.tile([128, 1024], f32, tag="s")
                    for ct in range(OC // 128):
                        nc.tensor.matmul(
                            ps[:, 0:512],
                            lhsT=OT[:, ct, tq * 128:(tq + 1) * 128],
                            rhs=WO[:, ct, oc2 * 512:(oc2 + 1) * 512],
                            start=(ct == 0), stop=(ct == OC // 128 - 1),
                        )
                    ob = ostage.tile([128, 512], f32, tag="ob")
                    nc.vector.tensor_tensor(
                        out=ob[:], in0=ps[:, 0:512],
                        in1=BO[:, oc2 * 512:(oc2 + 1) * 512],
                        op=mybir.AluOpType.add,
                    )
                    nc.sync.dma_start(
                        out=out[tq * 128:(tq + 1) * 128, oc2 * 512:(oc2 + 1) * 512],
                        in_=ob[:],
                    )

            if debug:
                nc.sync.dma_start(
                    out=d_qkt[:, :], in_=QKT[:].rearrange("p a t -> p (a t)"))
                nc.sync.dma_start(
                    out=d_vp[:, :], in_=VP[:].rearrange("p a b c -> p (a b c)"))
                nc.sync.dma_start(
                    out=d_osb[:, :], in_=OSB[:].rearrange("p a b c -> p (a b c)"))
                nc.sync.dma_start(
                    out=d_ot[:, :], in_=OT[:].rearrange("p a t -> p (a t)"))

    nc.compile()
    return nc


def _in_maps(x, W_qkv, b_qkv, W_o, b_o):
    x = np.asarray(x, np.float32)
    W_qkv = np.asarray(W_qkv, np.float32)
    b_qkv = np.asarray(b_qkv, np.float32)
    W_o = np.asarray(W_o, np.float32)
    b_o = np.asarray(b_o, np.float32)

    maps = []
    for c in range(N_CORES):
        b, hh = c // 2, c % 2
        rs = slice(hh * OC, (hh + 1) * OC)
        wq = W_qkv[0 * D:1 * D][rs]            # [512, 1024]
        wk = W_qkv[1 * D:2 * D][rs]
        wv = W_qkv[2 * D:3 * D][rs]
        wqkT = np.concatenate([wq, wk], 0).T   # [1024, 1024]
        bq = b_qkv[0 * D:1 * D][rs]
        bk = b_qkv[1 * D:2 * D][rs]
        bvv = b_qkv[2 * D:3 * D][rs]
        tri = np.triu(np.ones((128, 128), np.float32))
        maps.append({
            "xT": np.ascontiguousarray(x[b].T).astype(BF16),
            "wqkT": np.ascontiguousarray(wqkT).astype(BF16),
            "wvT": np.ascontiguousarray(wv.T).astype(BF16),
            "woT": np.ascontiguousarray(W_o[:, rs].T).astype(BF16),
            "bqk": np.concatenate([bq, bk]).reshape(2 * OC, 1),
            "bv": bvv.reshape(1, OC),
            "bo": (0.5 * b_o).reshape(1, D),
            "tri": tri.astype(BF16),
        })
    return maps


def _run(x, W_qkv, b_qkv, W_o, b_o, trace=False, tmpdir=None):
    from concourse.bass_utils import run_bass_kernel_spmd

    if "nc" not in _cache:
        _cache["nc"] = _build()
    res = run_bass_kernel_spmd(
        _cache["nc"], _in_maps(x, W_qkv, b_qkv, W_o, b_o),
        core_ids=list(range(N_CORES)), trace=trace, tmpdir=tmpdir,
    )
    out = np.empty((B, T, D), np.float32)
    for b in range(B):
        out[b] = res.results[2 * b]["out"] + res.results[2 * b + 1]["out"]
    return out, res


def kernel(x, W_qkv, b_qkv, W_o, b_o):
    out, _ = _run(x, W_qkv, b_qkv, W_o, b_o, trace=False)
    return out
